# revision 11
# baseline (speedup 1.0000x reference)
"""ChunkedDiagonalMLP Trainium2 kernel — 8-core SPMD, data-parallel over tokens.

Math (per token row x of width 4096, split into 8 chunks of 512):
    h_n  = gelu(x_n @ w1[n] + b1[n])          (exact erf gelu)
    y_n  = h_n @ w2[n] + b2[n]
    out  = LayerNorm(concat_n(y_n) + x) * ln_g + ln_b

Per core (2048 tokens), software-pipelined over 32 chunk-blocks (4 token
groups x 8 chunks), L2 lagging L1 by one block so the PE never waits on
the gelu handoff:
  - layer 1 feature-major: stationary w1 blocks, moving x^T (host-pre-
    transposed, bf16) -> h^T in PSUM, gelu+bias fused on ScalarE -> bf16 h^T
  - layer 2 token-major: stationary h^T blocks, moving w2 -> y in PSUM
  - residual from a bf16 token-major x read, fused with partial mean
    accumulation (scalar_tensor_tensor accum_out on DVE); sum-of-squares
    via a second stt (osl*osl) also on DVE (bf16 2x mode)
  - LayerNorm stats per group; apply split across DVE (tensor_scalar) and
    ScalarE (Copy with scale=rstd, bias=-mu*rstd); output stored bf16 and
    cast to fp32 on host
  - DMA queues: SP ring = x inputs, Act ring = weights (front-loaded),
    SWDGE (Pool) = output stores, so input prefetch never queues behind
    stores and the PE starts ~2us into the exec
"""

import numpy as np
import ml_dtypes
from contextlib import ExitStack

import concourse.bass as bass
import concourse.mybir as mybir
import concourse.tile as tile
from concourse.bass_utils import run_bass_kernel_spmd

N_CORES = 8
D = 4096
NCH = 8          # chunks
CH = 512         # chunk width
KT = CH // 128   # k-tiles per chunk (4)
S = 2048         # tokens per core
TG = 512         # tokens per group
NG = S // TG     # 4 groups
IT = TG // 128   # 128-token tiles per group (4)
EPS = 1e-5

F32 = mybir.dt.float32
BF16 = mybir.dt.bfloat16
F8 = mybir.dt.float8e4
BF = ml_dtypes.bfloat16
NPF8 = ml_dtypes.float8_e4m3
W2_SCALE = 256.0  # keeps fp8 w2 out of the subnormal range; undone in the stt
FP8 = True       # fp8e4 DoubleRow for layer 2 (bf16 fallback when False)
FP8_L1 = True    # fp8e4 DoubleRow for layer 1 (x and w1 quantized on host)
W1_SCALE = 256.0  # keeps fp8 w1 out of the subnormal range; undone in the gelu


def _split_excess_waits(nc, limit=1):
    """walrus CoreV3 codegen rejects instructions with too many sem waits
    (Drain allows only 1); move extras onto preceding same-engine NoOps."""
    n_split = 0
    for bb in nc.main_func.blocks:
        new_insts = []
        changed = False
        for inst in bb.instructions:
            lim = limit
            si = inst.sync_info
            if si is not None and si.on_wait and len(si.on_wait) > lim:
                waits = list(si.on_wait)
                extra, keep = waits[:-lim], waits[-lim:]
                for i in range(0, len(extra), lim):
                    nop = mybir.InstNoOp(
                        name=f"{inst.name}-ws{i}",
                        engine=inst.engine,
                        ins=[],
                        outs=[],
                        sync_info=mybir.SyncInfo(
                            on_wait=list(extra[i : i + lim]), on_update=[]
                        ),
                    )
                    new_insts.append(nop)
                    n_split += 1
                inst.sync_info = mybir.SyncInfo(
                    on_wait=list(keep), on_update=list(si.on_update)
                )
                changed = True
            new_insts.append(inst)
        if changed:
            bb.instructions[:] = new_insts
    return n_split


def _build(use_b2, use_lng, use_lnb, reps=1, x_bufs=4, h_bufs=3, ph_bufs=3,
           py_bufs=5, o_bufs=2, pf=2, tail_groups=(256, 256),
           store_rings="gsyg", tail_store_rings="gs"):
    # token groups: bulk 512-wide, then a tapered tail so the final
    # stats/apply/store chain covers few tokens and drains fast
    groups = []
    pos = 0
    tail_total = sum(tail_groups)
    while pos < S - tail_total:
        groups.append((pos, TG))
        pos += TG
    for t in tail_groups:
        groups.append((pos, t))
        pos += t
    assert pos == S, (groups,)
    nc = bass.Bass()
    # x^T per core: [n, k, c(128), t] fp8 (layer-1 input only; the residual
    # path reads the separate bf16 token-major copy)
    xT_e = nc.declare_dram_parameter(
        "xT", [NCH, KT, 128, S], F8 if FP8_L1 else BF16, isOutput=False
    )
    # token-major x rows (bf16) for residual
    xr_e = nc.declare_dram_parameter("xr", [S, D], BF16, isOutput=False)
    # weights: [n, c(128), k, d] (host pre-permuted so partition lines
    # are 4KB contiguous)
    w1_e = nc.declare_dram_parameter(
        "w1", [NCH, 128, KT, CH], F8 if FP8_L1 else BF16, isOutput=False
    )
    w2_e = nc.declare_dram_parameter(
        "w2", [NCH, 128, KT, CH], F8 if FP8 else BF16, isOutput=False
    )
    # b1 rearranged to [128, n*4+j] columns
    b1_e = nc.declare_dram_parameter("b1c", [128, NCH * KT], F32, isOutput=False)
    b2_e = nc.declare_dram_parameter("b2", [NCH, CH], F32, isOutput=False)
    lng_e = nc.declare_dram_parameter("ln_g", [D], F32, isOutput=False)
    lnb_e = nc.declare_dram_parameter("ln_b", [D], F32, isOutput=False)
    out_e = nc.declare_dram_parameter("out", [S, D], BF16, isOutput=True)

    with tile.TileContext(nc) as tc:
        with ExitStack() as ctx:
            opool = ctx.enter_context(tc.tile_pool(name="opool", bufs=o_bufs))
            xpool = ctx.enter_context(tc.tile_pool(name="xpool", bufs=x_bufs))
            hpool = ctx.enter_context(tc.tile_pool(name="hpool", bufs=h_bufs))
            spool = ctx.enter_context(tc.tile_pool(name="spool", bufs=2))
            cpool = ctx.enter_context(tc.tile_pool(name="cpool", bufs=1))
            pp_h = ctx.enter_context(tc.tile_pool(name="pp_h", bufs=ph_bufs, space="PSUM"))
            pp_y = ctx.enter_context(tc.tile_pool(name="pp_y", bufs=py_bufs, space="PSUM"))

            # ---- constants / weights (resident), all on the Act HWDGE ring
            # so the SP ring starts the first xT loads immediately ----
            b1_sb = cpool.tile([128, NCH * KT], F32)
            nc.scalar.dma_start(out=b1_sb, in_=b1_e[:, :])
            eps_sb = cpool.tile([128, 1], F32)
            nc.vector.memset(eps_sb, EPS)

            # Weight tiles are declared up front but DMA'd just-in-time from
            # inside the first group's block loop so the start of the kernel
            # isn't DMA-bound on 8MB of weights.
            w1_sb = [
                cpool.tile([128, KT, CH], F8 if FP8_L1 else BF16, name=f"w1_{n}")
                for n in range(NCH)
            ]
            w2_sb = [
                cpool.tile([128, KT, CH], F8 if FP8 else BF16, name=f"w2_{n}")
                for n in range(NCH)
            ]
            w_loaded = set()

            def load_w(which, n):
                if n < NCH and (which, n) not in w_loaded:
                    w_loaded.add((which, n))
                    t, e = (w1_sb, w1_e) if which == 1 else (w2_sb, w2_e)
                    nc.scalar.dma_start(out=t[n], in_=e[n])

            load_w(1, 0)
            load_w(2, 0)
            load_w(1, 1)

            b2_sb = None
            if use_b2:
                b2_sb = cpool.tile([128, NCH, CH], F32)
                nc.gpsimd.dma_start(
                    out=b2_sb,
                    in_=bass.AP(
                        tensor=b2_e.tensor,
                        offset=b2_e.offset,
                        ap=[[0, 128], b2_e.ap[0], b2_e.ap[1]],
                    ),
                )
            lng_sb = None
            if use_lng:
                lng_sb = cpool.tile([128, D], F32)
                nc.gpsimd.dma_start(
                    out=lng_sb,
                    in_=bass.AP(
                        tensor=lng_e.tensor, offset=lng_e.offset,
                        ap=[[0, 128], lng_e.ap[0]],
                    ),
                )
            lnb_sb = None
            if use_lnb:
                lnb_sb = cpool.tile([128, D], F32)
                nc.gpsimd.dma_start(
                    out=lnb_sb,
                    in_=bass.AP(
                        tensor=lnb_e.tensor, offset=lnb_e.offset,
                        ap=[[0, 128], lnb_e.ap[0]],
                    ),
                )

            for rep in range(reps):
                blocks = [(g, n) for g in range(len(groups)) for n in range(NCH)]
                xT_tiles = {}
                xr_tiles = {}
                hT_tiles = {}
                gstate = {}

                def emit_xdma(idx):
                    g, n = blocks[idx]
                    gpos, gt = groups[g]
                    tsl = slice(gpos, gpos + gt)
                    xT_sb = xpool.tile(
                        [128, KT, gt], F8 if FP8_L1 else BF16, name="xT_sb",
                        bufs=pf + 2,
                    )
                    nc.sync.dma_start(
                        out=xT_sb,
                        in_=xT_e[n, :, :, tsl].rearrange("k c t -> c k t"),
                    )
                    xT_tiles[idx] = xT_sb
                    xr_sb = xpool.tile(
                        [128, gt // 128, CH], BF16, name="xr_sb", bufs=pf + 3
                    )
                    nc.sync.dma_start(
                        out=xr_sb,
                        in_=xr_e[tsl, n * CH : (n + 1) * CH].rearrange(
                            "(i p) d -> p i d", p=128
                        ),
                    )
                    xr_tiles[idx] = xr_sb

                def emit_L1(idx):
                    g, n = blocks[idx]
                    gt = groups[g][1]
                    xT_sb = xT_tiles.pop(idx)
                    hT = hpool.tile([128, KT, gt], F8 if FP8 else BF16, name="hT")
                    hT_tiles[idx] = hT
                    for j in range(KT):
                        ph = pp_h.tile([128, gt], F32, tag="ph", name="ph")
                        if FP8_L1:
                            # fp8 DoubleRow: each matmul contracts 2 k-tiles
                            for a in range(KT // 2):
                                nc.tensor.matmul(
                                    ph,
                                    w1_sb[n][:, 2 * a : 2 * a + 2, j * 128 : (j + 1) * 128],
                                    xT_sb[:, 2 * a : 2 * a + 2, :],
                                    start=(a == 0),
                                    stop=(a == KT // 2 - 1),
                                    perf_mode=mybir.MatmulPerfMode.DoubleRow,
                                )
                        else:
                            for k in range(KT):
                                nc.tensor.matmul(
                                    ph,
                                    w1_sb[n][:, k, j * 128 : (j + 1) * 128],
                                    xT_sb[:, k, :],
                                    start=(k == 0),
                                    stop=(k == KT - 1),
                                )
                        nc.scalar.activation(
                            out=hT[:, j, :],
                            in_=ph,
                            func=mybir.ActivationFunctionType.Gelu,
                            bias=b1_sb[:, n * KT + j : n * KT + j + 1],
                            scale=(1.0 / W1_SCALE) if FP8_L1 else 1.0,
                        )

                def emit_stats_apply_i(g, i):
                    with tc.high_priority():
                        _emit_stats_apply_i(g, i)

                def _emit_stats_apply_i(g, i):
                    sums, sqs, outs = gstate[g]
                    gpos, gt = groups[g]
                    tsl = slice(gpos, gpos + gt)
                    mu = spool.tile([128, 1], F32, name="mu", bufs=4)
                    nc.vector.tensor_reduce(
                        out=mu, in_=sums[:, i, :], axis=mybir.AxisListType.X,
                        op=mybir.AluOpType.add,
                    )
                    nc.vector.tensor_scalar_mul(out=mu, in0=mu, scalar1=1.0 / D)
                    # var = E[x^2] - mu^2 ; rstd = 1/sqrt(var + eps)
                    var = spool.tile([128, 1], F32, name="var", bufs=4)
                    nc.vector.tensor_reduce(
                        out=var, in_=sqs[:, i, :], axis=mybir.AxisListType.X,
                        op=mybir.AluOpType.add,
                    )
                    nc.vector.tensor_scalar_mul(out=var, in0=var, scalar1=1.0 / D)
                    mu2 = spool.tile([128, 1], F32, name="mu2", bufs=4)
                    nc.vector.tensor_mul(out=mu2, in0=mu, in1=mu)
                    nc.vector.tensor_sub(out=var, in0=var, in1=mu2)
                    rs = spool.tile([128, 1], F32, name="rs", bufs=4)
                    nc.scalar.activation(
                        out=rs, in_=var,
                        func=mybir.ActivationFunctionType.Sqrt,
                        bias=eps_sb,
                    )
                    nc.vector.reciprocal(out=rs, in_=rs)
                    # LN apply on Pool: frees DVE for the stt accumulate chain
                    # (DVE would otherwise exceed the fp8 PE roofline)
                    nc.gpsimd.tensor_scalar(
                        out=outs[i],
                        in0=outs[i],
                        scalar1=mu,
                        scalar2=rs,
                        op0=mybir.AluOpType.subtract,
                        op1=mybir.AluOpType.mult,
                    )
                    if use_lng:
                        nc.gpsimd.tensor_mul(out=outs[i], in0=outs[i], in1=lng_sb)
                    if use_lnb:
                        nc.gpsimd.tensor_add(out=outs[i], in0=outs[i], in1=lnb_sb)
                    se_pat = store_rings if g < len(groups) - len(tail_groups) else tail_store_rings
                    store_eng = {"g": nc.gpsimd, "s": nc.scalar, "y": nc.sync}[
                        se_pat[i % len(se_pat)]
                    ]
                    store_eng.dma_start(
                        out=out_e[tsl, :].rearrange("(i p) d -> p i d", p=128)[
                            :, i, :
                        ],
                        in_=outs[i],
                    )

                def emit_L2(idx):
                    g, n = blocks[idx]
                    git = groups[g][1] // 128
                    if n == 0:
                        sums = spool.tile([128, git, NCH], F32, name="sums")
                        sqs = spool.tile([128, git, NCH], F32, name="sqs")
                        outs = [
                            opool.tile([128, D], BF16, name=f"o{i}")
                            for i in range(git)
                        ]
                        gstate[g] = (sums, sqs, outs)
                    sums, sqs, outs = gstate[g]
                    hT = hT_tiles.pop(idx)
                    xr_sb = xr_tiles.pop(idx)
                    for i in range(git):
                        py = pp_y.tile([128, CH], F32, tag="py", name="py")
                        if FP8:
                            # fp8 DoubleRow: each matmul contracts 2 k-tiles
                            for a in range(KT // 2):
                                nc.tensor.matmul(
                                    py,
                                    hT[:, 2 * a : 2 * a + 2, i * 128 : (i + 1) * 128],
                                    w2_sb[n][:, 2 * a : 2 * a + 2, :],
                                    start=(a == 0),
                                    stop=(a == KT // 2 - 1),
                                    perf_mode=mybir.MatmulPerfMode.DoubleRow,
                                )
                        else:
                            for j in range(KT):
                                nc.tensor.matmul(
                                    py,
                                    hT[:, j, i * 128 : (i + 1) * 128],
                                    w2_sb[n][:, j, :],
                                    start=(j == 0),
                                    stop=(j == KT - 1),
                                )
                        osl = outs[i][:, n * CH : (n + 1) * CH]
                        # out = y/W2_SCALE + x ; accumulate per-token partial sum
                        nc.vector.scalar_tensor_tensor(
                            out=osl,
                            in0=py,
                            scalar=(1.0 / W2_SCALE) if FP8 else 1.0,
                            in1=xr_sb[:, i, :],
                            op0=mybir.AluOpType.mult,
                            op1=mybir.AluOpType.add,
                            accum_out=sums[:, i, n : n + 1],
                        )
                        if use_b2:
                            nc.vector.tensor_add(osl, osl, b2_sb[:, n, :])
                        # sum of squares on DVE (bf16 2x mode)
                        sq = spool.tile([128, CH], BF16, tag="sq", name="sq")
                        nc.vector.scalar_tensor_tensor(
                            out=sq,
                            in0=osl,
                            scalar=1.0,
                            in1=osl,
                            op0=mybir.AluOpType.mult,
                            op1=mybir.AluOpType.mult,
                            accum_out=sqs[:, i, n : n + 1],
                        )
                        # last chunk: this i-tile's LN stats are complete —
                        # normalize and store it while L2 continues
                        if n == NCH - 1:
                            emit_stats_apply_i(g, i)
                    if n == NCH - 1:
                        gstate.pop(g)

                for idx in range(min(pf, len(blocks))):
                    emit_xdma(idx)
                for idx, (g, n) in enumerate(blocks):
                    if rep == 0 and g == 0:
                        load_w(1, n + 2)
                        load_w(2, n + 1)
                    if idx + pf < len(blocks):
                        emit_xdma(idx + pf)
                    emit_L1(idx)
                    if idx >= 1:
                        emit_L2(idx - 1)
                emit_L2(len(blocks) - 1)

    _split_excess_waits(nc)
    return nc


_CACHE = {}


def prep_inputs(inputs):
    """Host-side sharding + layout prep -> per-core input maps."""
    x = np.asarray(inputs["x"])
    w1 = np.asarray(inputs["w1"], dtype=np.float32)
    w2 = np.asarray(inputs["w2"], dtype=np.float32)
    b1 = np.asarray(inputs["b1"], dtype=np.float32)
    b2 = np.asarray(inputs["b2"], dtype=np.float32)
    ln_g = np.asarray(inputs["ln_g"], dtype=np.float32)
    ln_b = np.asarray(inputs["ln_b"], dtype=np.float32)
    B, L, d = x.shape
    x2 = np.ascontiguousarray(x.reshape(B * L, D).astype(np.float32))
    _w1p = w1.reshape(NCH, KT, 128, CH).transpose(0, 2, 1, 3)
    w1h = np.ascontiguousarray(
        (_w1p * W1_SCALE).astype(NPF8) if FP8_L1 else _w1p.astype(BF)
    )
    _w2p = w2.reshape(NCH, KT, 128, CH).transpose(0, 2, 1, 3)
    w2h = np.ascontiguousarray(
        (_w2p * W2_SCALE).astype(NPF8) if FP8 else _w2p.astype(BF)
    )
    b1h = np.ascontiguousarray(
        b1.reshape(NCH, KT, 128).transpose(2, 0, 1).reshape(128, NCH * KT)
    )

    in_maps = []
    for c in range(N_CORES):
        rows = x2[c * S : (c + 1) * S]  # [S, D] fp32
        xTh = (
            np.ascontiguousarray(rows.T)
            .astype(NPF8 if FP8_L1 else BF)
            .reshape(NCH, KT, 128, S)
        )
        in_maps.append(
            {
                "xT": xTh,
                "xr": rows.astype(BF),
                "w1": w1h,
                "w2": w2h,
                "b1c": b1h,
                "b2": b2,
                "ln_g": ln_g,
                "ln_b": ln_b,
            }
        )
    return in_maps


def kernel(x, w1, b1, w2, b2, ln_g, ln_b):
    x = np.asarray(x)
    b2 = np.asarray(b2, dtype=np.float32)
    ln_g = np.asarray(ln_g, dtype=np.float32)
    ln_b = np.asarray(ln_b, dtype=np.float32)
    B, L, d = x.shape
    assert d == D and B * L == N_CORES * S, (x.shape,)

    use_b2 = bool(np.any(b2 != 0.0))
    use_lng = bool(np.any(ln_g != 1.0))
    use_lnb = bool(np.any(ln_b != 0.0))

    key = (use_b2, use_lng, use_lnb)
    if key not in _CACHE:
        _CACHE[key] = _build(*key)
    nc = _CACHE[key]

    in_maps = prep_inputs(
        {"x": x, "w1": w1, "b1": b1, "w2": w2, "b2": b2, "ln_g": ln_g, "ln_b": ln_b}
    )

    res = run_bass_kernel_spmd(nc, in_maps, list(range(N_CORES)))
    out = np.concatenate([res.results[c]["out"] for c in range(N_CORES)], axis=0)
    return out.reshape(B, L, D).astype(np.float32)



# revision 15
# speedup vs baseline: 5.1175x; 5.1175x over previous
"""ChunkedDiagonalMLP Trainium2 kernel — 8-core SPMD, data-parallel over tokens.

Math (per token row x of width 4096, split into 8 chunks of 512):
    h_n  = gelu(x_n @ w1[n] + b1[n])          (exact erf gelu)
    y_n  = h_n @ w2[n] + b2[n]
    out  = LayerNorm(concat_n(y_n) + x) * ln_g + ln_b

Per core (2048 tokens), software-pipelined over 16 chunk-PAIR blocks
(4 token groups x 4 pairs), L2 lagging L1 by one pair so the PE never
waits on the gelu handoff:
  - both layers fp8e4 DoubleRow (x, w1, h, w2 quantized; scales undone
    downstream); PE work = 2 matmuls per 256-wide contraction
  - layer 1 feature-major: stationary w1, moving x^T (host-pre-transposed
    fp8) -> h^T in PSUM; gelu reads a 2-bank [128,2,512] PSUM tile in ONE
    ScalarE instruction (b1==0 fast path) -> fp8 h^T
  - layer 2 token-major: chunk pairs share a 2-bank [128,1024] PSUM tile;
    ONE DVE scalar_tensor_tensor per pair does y/W2_SCALE + x (residual,
    bf16 token-major x) with accum_out -> per-pair token sums
  - LN stats: per 128-token tile, one batched sum-of-squares pass over
    [128,4096] (DVE stt or ScalarE Square+accum_out, split to balance
    engines); sqrt batched per group (one act-table swap per group);
    apply on DVE tensor_scalar (4x mode on bf16)
  - DMA queues: SP ring = x inputs, Act ring = weights (JIT-loaded),
    stores alternate SWDGE (Pool) / SP so input prefetch never queues
    behind stores
"""

import numpy as np
import ml_dtypes
from contextlib import ExitStack

import concourse.bass as bass
import concourse.mybir as mybir
import concourse.tile as tile
from concourse.bass_utils import run_bass_kernel_spmd

N_CORES = 8
D = 4096
NCH = 8          # chunks
NP = NCH // 2    # chunk pairs (4)
CH = 512         # chunk width
KT = CH // 128   # k-tiles per chunk (4)
S = 2048         # tokens per core
TG = 512         # tokens per group
NG = S // TG     # 4 groups
GIT = TG // 128  # 128-token tiles per group (4)
EPS = 1e-5

F32 = mybir.dt.float32
BF16 = mybir.dt.bfloat16
F8 = mybir.dt.float8e4
BF = ml_dtypes.bfloat16
NPF8 = ml_dtypes.float8_e4m3
W2_SCALE = 256.0  # keeps fp8 w2 out of the subnormal range; undone in the stt
FP8 = True       # fp8e4 DoubleRow for layer 2 (bf16 fallback when False)
FP8_L1 = True    # fp8e4 DoubleRow for layer 1 (x and w1 quantized on host)
W1_SCALE = 256.0  # keeps fp8 w1 out of the subnormal range; undone in the gelu
DR = mybir.MatmulPerfMode.DoubleRow


def _split_excess_waits(nc, limit=1):
    """walrus CoreV3 codegen rejects instructions with too many sem waits
    (Drain allows only 1); move extras onto preceding same-engine NoOps."""
    n_split = 0
    for bb in nc.main_func.blocks:
        new_insts = []
        changed = False
        for inst in bb.instructions:
            lim = limit
            si = inst.sync_info
            if si is not None and si.on_wait and len(si.on_wait) > lim:
                waits = list(si.on_wait)
                extra, keep = waits[:-lim], waits[-lim:]
                for i in range(0, len(extra), lim):
                    nop = mybir.InstNoOp(
                        name=f"{inst.name}-ws{i}",
                        engine=inst.engine,
                        ins=[],
                        outs=[],
                        sync_info=mybir.SyncInfo(
                            on_wait=list(extra[i : i + lim]), on_update=[]
                        ),
                    )
                    new_insts.append(nop)
                    n_split += 1
                inst.sync_info = mybir.SyncInfo(
                    on_wait=list(keep), on_update=list(si.on_update)
                )
                changed = True
            new_insts.append(inst)
        if changed:
            bb.instructions[:] = new_insts
    return n_split


def _build(use_b2, use_lng, use_lnb, reps=1, x_bufs=4, h_bufs=3, ph_bufs=2,
           py_bufs=2, o_bufs=2, pf=2, sq_dve_mod=4, use_b1=False,
           store_rings="gy"):
    groups = [(gi * TG, TG) for gi in range(NG)]
    nc = bass.Bass()
    # x^T per core: [n, k, c(128), t] fp8 (layer-1 input only; the residual
    # path reads the separate bf16 token-major copy)
    xT_e = nc.declare_dram_parameter(
        "xT", [NCH, KT, 128, S], F8 if FP8_L1 else BF16, isOutput=False
    )
    # token-major x rows (bf16) for residual
    xr_e = nc.declare_dram_parameter("xr", [S, D], BF16, isOutput=False)
    # weights: [n, c(128), k, d] (host pre-permuted so partition lines
    # are contiguous)
    w1_e = nc.declare_dram_parameter(
        "w1", [NCH, 128, KT, CH], F8 if FP8_L1 else BF16, isOutput=False
    )
    w2_e = nc.declare_dram_parameter(
        "w2", [NCH, 128, KT, CH], F8 if FP8 else BF16, isOutput=False
    )
    # b1 rearranged to [128, n*4+j] columns
    b1_e = nc.declare_dram_parameter("b1c", [128, NCH * KT], F32, isOutput=False)
    b2_e = nc.declare_dram_parameter("b2", [NCH, CH], F32, isOutput=False)
    lng_e = nc.declare_dram_parameter("ln_g", [D], F32, isOutput=False)
    lnb_e = nc.declare_dram_parameter("ln_b", [D], F32, isOutput=False)
    out_e = nc.declare_dram_parameter("out", [S, D], BF16, isOutput=True)

    with tile.TileContext(nc) as tc:
        with ExitStack() as ctx:
            opool = ctx.enter_context(tc.tile_pool(name="opool", bufs=o_bufs))
            xpool = ctx.enter_context(tc.tile_pool(name="xpool", bufs=x_bufs))
            hpool = ctx.enter_context(tc.tile_pool(name="hpool", bufs=h_bufs))
            spool = ctx.enter_context(tc.tile_pool(name="spool", bufs=2))
            cpool = ctx.enter_context(tc.tile_pool(name="cpool", bufs=1))
            pp_h = ctx.enter_context(tc.tile_pool(name="pp_h", bufs=ph_bufs, space="PSUM"))
            pp_y = ctx.enter_context(tc.tile_pool(name="pp_y", bufs=py_bufs, space="PSUM"))

            # ---- constants / weights (resident), all on the Act HWDGE ring
            # so the SP ring starts the first xT loads immediately ----
            b1_sb = cpool.tile([128, NCH * KT], F32)
            nc.scalar.dma_start(out=b1_sb, in_=b1_e[:, :])
            eps_sb = cpool.tile([128, 1], F32)
            nc.vector.memset(eps_sb, EPS)

            # Weight tiles are declared up front but DMA'd just-in-time from
            # inside the first group's pair loop so the start of the kernel
            # isn't DMA-bound on the weights.
            w1_sb = [
                cpool.tile([128, KT, CH], F8 if FP8_L1 else BF16, name=f"w1_{n}")
                for n in range(NCH)
            ]
            w2_sb = [
                cpool.tile([128, KT, CH], F8 if FP8 else BF16, name=f"w2_{n}")
                for n in range(NCH)
            ]
            w_loaded = set()

            def load_w(which, n):
                if n < NCH and (which, n) not in w_loaded:
                    w_loaded.add((which, n))
                    t, e = (w1_sb, w1_e) if which == 1 else (w2_sb, w2_e)
                    nc.scalar.dma_start(out=t[n], in_=e[n])

            load_w(1, 0)
            load_w(1, 1)
            load_w(2, 0)
            load_w(2, 1)
            load_w(1, 2)
            load_w(1, 3)

            b2_sb = None
            if use_b2:
                b2_sb = cpool.tile([128, NCH, CH], F32)
                nc.gpsimd.dma_start(
                    out=b2_sb,
                    in_=bass.AP(
                        tensor=b2_e.tensor,
                        offset=b2_e.offset,
                        ap=[[0, 128], b2_e.ap[0], b2_e.ap[1]],
                    ),
                )
            lng_sb = None
            if use_lng:
                lng_sb = cpool.tile([128, D], F32)
                nc.gpsimd.dma_start(
                    out=lng_sb,
                    in_=bass.AP(
                        tensor=lng_e.tensor, offset=lng_e.offset,
                        ap=[[0, 128], lng_e.ap[0]],
                    ),
                )
            lnb_sb = None
            if use_lnb:
                lnb_sb = cpool.tile([128, D], F32)
                nc.gpsimd.dma_start(
                    out=lnb_sb,
                    in_=bass.AP(
                        tensor=lnb_e.tensor, offset=lnb_e.offset,
                        ap=[[0, 128], lnb_e.ap[0]],
                    ),
                )

            for rep in range(reps):
                pairs = [(g, p) for g in range(NG) for p in range(NP)]
                xT_tiles = {}
                xr_tiles = {}
                hT_tiles = {}
                gstate = {}

                def emit_xdma(idx):
                    g, p = pairs[idx]
                    n0 = 2 * p
                    gpos, gt = groups[g]
                    tsl = slice(gpos, gpos + gt)
                    xT_sb = xpool.tile(
                        [128, 2, KT, gt], F8 if FP8_L1 else BF16, name="xT_sb",
                        bufs=pf + 2,
                    )
                    nc.sync.dma_start(
                        out=xT_sb,
                        in_=xT_e[n0 : n0 + 2, :, :, tsl].rearrange(
                            "n k c t -> c n k t"
                        ),
                    )
                    xT_tiles[idx] = xT_sb
                    xr_sb = xpool.tile(
                        [128, GIT, 2 * CH], BF16, name="xr_sb", bufs=pf + 2
                    )
                    nc.sync.dma_start(
                        out=xr_sb,
                        in_=xr_e[tsl, n0 * CH : (n0 + 2) * CH].rearrange(
                            "(i p) d -> p i d", p=128
                        ),
                    )
                    xr_tiles[idx] = xr_sb

                def emit_L1(idx):
                    g, p = pairs[idx]
                    gt = groups[g][1]
                    xT_sb = xT_tiles.pop(idx)
                    hT = hpool.tile(
                        [128, 2, KT, gt], F8 if FP8 else BF16, name="hT"
                    )
                    hT_tiles[idx] = hT
                    for nn in range(2):
                        n = 2 * p + nn
                        for jp in range(KT // 2):
                            ph = pp_h.tile([128, 2, gt], F32, tag="ph", name="ph")
                            for jj in range(2):
                                j = 2 * jp + jj
                                if FP8_L1:
                                    for a in range(KT // 2):
                                        nc.tensor.matmul(
                                            ph[:, jj, :],
                                            w1_sb[n][:, 2 * a : 2 * a + 2,
                                                     j * 128 : (j + 1) * 128],
                                            xT_sb[:, nn, 2 * a : 2 * a + 2, :],
                                            start=(a == 0),
                                            stop=(a == KT // 2 - 1),
                                            perf_mode=DR,
                                        )
                                else:
                                    for k in range(KT):
                                        nc.tensor.matmul(
                                            ph[:, jj, :],
                                            w1_sb[n][:, k, j * 128 : (j + 1) * 128],
                                            xT_sb[:, nn, k, :],
                                            start=(k == 0),
                                            stop=(k == KT - 1),
                                        )
                            if use_b1:
                                # general path: per-j bias columns
                                for jj in range(2):
                                    j = 2 * jp + jj
                                    nc.scalar.activation(
                                        out=hT[:, nn, j, :],
                                        in_=ph[:, jj, :],
                                        func=mybir.ActivationFunctionType.Gelu,
                                        bias=b1_sb[:, n * KT + j : n * KT + j + 1],
                                        scale=(1.0 / W1_SCALE) if FP8_L1 else 1.0,
                                    )
                            else:
                                # b1 == 0: one gelu over the 2-bank PSUM tile
                                nc.scalar.activation(
                                    out=hT[:, nn, 2 * jp : 2 * jp + 2, :],
                                    in_=ph,
                                    func=mybir.ActivationFunctionType.Gelu,
                                    bias=0.0,
                                    scale=(1.0 / W1_SCALE) if FP8_L1 else 1.0,
                                )

                def emit_sq_stats(g, i):
                    """after outs[i] complete: batched sum-of-squares pass,
                    then mu/var smalls; engine for squares alternates to
                    balance DVE vs ScalarE."""
                    with tc.high_priority():
                        _emit_sq_stats(g, i)

                def _emit_sq_stats(g, i):
                    sums, sqs, mus, gvar, outs = gstate[g]
                    sqb = spool.tile([128, D], BF16, tag="sqb", name="sqb", bufs=2)
                    i_glob = g * GIT + i
                    if i_glob % sq_dve_mod == 0:
                        nc.vector.scalar_tensor_tensor(
                            out=sqb,
                            in0=outs[i],
                            scalar=1.0,
                            in1=outs[i],
                            op0=mybir.AluOpType.mult,
                            op1=mybir.AluOpType.mult,
                            accum_out=sqs[:, i : i + 1],
                        )
                    else:
                        nc.scalar.activation(
                            out=sqb,
                            in_=outs[i],
                            func=mybir.ActivationFunctionType.Square,
                            accum_out=sqs[:, i : i + 1],
                        )
                    # mu = sum(sums_pairs)/D
                    nc.vector.tensor_reduce(
                        out=mus[:, i : i + 1], in_=sums[:, i, :],
                        axis=mybir.AxisListType.X, op=mybir.AluOpType.add,
                    )
                    nc.vector.tensor_scalar_mul(
                        out=mus[:, i : i + 1], in0=mus[:, i : i + 1],
                        scalar1=1.0 / D,
                    )
                    # var = sqs/D - mu^2
                    mu2 = spool.tile([128, 1], F32, name="mu2", bufs=4)
                    nc.vector.tensor_mul(
                        out=mu2, in0=mus[:, i : i + 1], in1=mus[:, i : i + 1]
                    )
                    nc.vector.scalar_tensor_tensor(
                        out=gvar[:, i : i + 1],
                        in0=sqs[:, i : i + 1],
                        scalar=1.0 / D,
                        in1=mu2,
                        op0=mybir.AluOpType.mult,
                        op1=mybir.AluOpType.subtract,
                    )

                def emit_apply_store(g):
                    with tc.high_priority():
                        _emit_apply_store(g)

                def _emit_apply_store(g):
                    sums, sqs, mus, gvar, outs = gstate[g]
                    gpos, gt = groups[g]
                    tsl = slice(gpos, gpos + gt)
                    git = gt // 128
                    # one sqrt for the whole group (single act-table swap)
                    srt = spool.tile([128, GIT], F32, name="srt", bufs=2)
                    nc.scalar.activation(
                        out=srt[:, :git], in_=gvar[:, :git],
                        func=mybir.ActivationFunctionType.Sqrt,
                        bias=eps_sb,
                    )
                    for i in range(git):
                        rs = spool.tile([128, 1], F32, name="rs", bufs=4)
                        nc.vector.reciprocal(out=rs, in_=srt[:, i : i + 1])
                        nc.vector.tensor_scalar(
                            out=outs[i],
                            in0=outs[i],
                            scalar1=mus[:, i : i + 1],
                            scalar2=rs,
                            op0=mybir.AluOpType.subtract,
                            op1=mybir.AluOpType.mult,
                        )
                        if use_lng:
                            nc.vector.tensor_mul(out=outs[i], in0=outs[i], in1=lng_sb)
                        if use_lnb:
                            nc.vector.tensor_add(out=outs[i], in0=outs[i], in1=lnb_sb)
                        store_eng = {"g": nc.gpsimd, "s": nc.scalar, "y": nc.sync}[
                            store_rings[i % len(store_rings)]
                        ]
                        store_eng.dma_start(
                            out=out_e[tsl, :].rearrange("(i p) d -> p i d", p=128)[
                                :, i, :
                            ],
                            in_=outs[i],
                        )

                def emit_L2(idx):
                    g, p = pairs[idx]
                    n0 = 2 * p
                    gt = groups[g][1]
                    git = gt // 128
                    if p == 0:
                        sums = spool.tile([128, GIT, NP], F32, name="sums")
                        sqs = spool.tile([128, GIT], F32, name="sqs")
                        mus = spool.tile([128, GIT], F32, name="mus")
                        gvar = spool.tile([128, GIT], F32, name="gvar")
                        outs = [
                            opool.tile([128, D], BF16, name=f"o{i}")
                            for i in range(git)
                        ]
                        gstate[g] = (sums, sqs, mus, gvar, outs)
                    sums, sqs, mus, gvar, outs = gstate[g]
                    hT = hT_tiles.pop(idx)
                    xr_sb = xr_tiles.pop(idx)
                    for i in range(git):
                        py = pp_y.tile([128, 2 * CH], F32, tag="py", name="py")
                        for nn in range(2):
                            n = n0 + nn
                            if FP8:
                                for a in range(KT // 2):
                                    nc.tensor.matmul(
                                        py[:, nn * CH : (nn + 1) * CH],
                                        hT[:, nn, 2 * a : 2 * a + 2,
                                           i * 128 : (i + 1) * 128],
                                        w2_sb[n][:, 2 * a : 2 * a + 2, :],
                                        start=(a == 0),
                                        stop=(a == KT // 2 - 1),
                                        perf_mode=DR,
                                    )
                            else:
                                for j in range(KT):
                                    nc.tensor.matmul(
                                        py[:, nn * CH : (nn + 1) * CH],
                                        hT[:, nn, j, i * 128 : (i + 1) * 128],
                                        w2_sb[n][:, j, :],
                                        start=(j == 0),
                                        stop=(j == KT - 1),
                                    )
                        osl = outs[i][:, n0 * CH : (n0 + 2) * CH]
                        # out = y/W2_SCALE + x ; accumulate per-token pair sums
                        nc.vector.scalar_tensor_tensor(
                            out=osl,
                            in0=py,
                            scalar=(1.0 / W2_SCALE) if FP8 else 1.0,
                            in1=xr_sb[:, i, :],
                            op0=mybir.AluOpType.mult,
                            op1=mybir.AluOpType.add,
                            accum_out=sums[:, i, p : p + 1],
                        )
                        if use_b2:
                            for nn in range(2):
                                nc.vector.tensor_add(
                                    outs[i][:, (n0 + nn) * CH : (n0 + nn + 1) * CH],
                                    outs[i][:, (n0 + nn) * CH : (n0 + nn + 1) * CH],
                                    b2_sb[:, n0 + nn, :],
                                )
                        # last pair: this i-tile is complete -> LN stats
                        if p == NP - 1:
                            emit_sq_stats(g, i)
                            if i == git - 1:
                                emit_apply_store(g)
                    if p == NP - 1:
                        gstate.pop(g)

                for idx in range(min(pf, len(pairs))):
                    emit_xdma(idx)
                for idx, (g, p) in enumerate(pairs):
                    if rep == 0 and g == 0:
                        n0 = 2 * p
                        load_w(1, n0 + 4)
                        load_w(1, n0 + 5)
                        load_w(2, n0 + 2)
                        load_w(2, n0 + 3)
                    if idx + pf < len(pairs):
                        emit_xdma(idx + pf)
                    emit_L1(idx)
                    if idx >= 1:
                        emit_L2(idx - 1)
                emit_L2(len(pairs) - 1)

    _split_excess_waits(nc)
    return nc


_CACHE = {}


def prep_inputs(inputs):
    """Host-side sharding + layout prep -> per-core input maps."""
    x = np.asarray(inputs["x"])
    w1 = np.asarray(inputs["w1"], dtype=np.float32)
    w2 = np.asarray(inputs["w2"], dtype=np.float32)
    b1 = np.asarray(inputs["b1"], dtype=np.float32)
    b2 = np.asarray(inputs["b2"], dtype=np.float32)
    ln_g = np.asarray(inputs["ln_g"], dtype=np.float32)
    ln_b = np.asarray(inputs["ln_b"], dtype=np.float32)
    B, L, d = x.shape
    x2 = np.ascontiguousarray(x.reshape(B * L, D).astype(np.float32))
    _w1p = w1.reshape(NCH, KT, 128, CH).transpose(0, 2, 1, 3)
    w1h = np.ascontiguousarray(
        (_w1p * W1_SCALE).astype(NPF8) if FP8_L1 else _w1p.astype(BF)
    )
    _w2p = w2.reshape(NCH, KT, 128, CH).transpose(0, 2, 1, 3)
    w2h = np.ascontiguousarray(
        (_w2p * W2_SCALE).astype(NPF8) if FP8 else _w2p.astype(BF)
    )
    b1h = np.ascontiguousarray(
        b1.reshape(NCH, KT, 128).transpose(2, 0, 1).reshape(128, NCH * KT)
    )

    in_maps = []
    for c in range(N_CORES):
        rows = x2[c * S : (c + 1) * S]  # [S, D] fp32
        xTh = (
            np.ascontiguousarray(rows.T)
            .astype(NPF8 if FP8_L1 else BF)
            .reshape(NCH, KT, 128, S)
        )
        in_maps.append(
            {
                "xT": xTh,
                "xr": rows.astype(BF),
                "w1": w1h,
                "w2": w2h,
                "b1c": b1h,
                "b2": b2,
                "ln_g": ln_g,
                "ln_b": ln_b,
            }
        )
    return in_maps


def kernel(x, w1, b1, w2, b2, ln_g, ln_b):
    x = np.asarray(x)
    b1 = np.asarray(b1, dtype=np.float32)
    b2 = np.asarray(b2, dtype=np.float32)
    ln_g = np.asarray(ln_g, dtype=np.float32)
    ln_b = np.asarray(ln_b, dtype=np.float32)
    B, L, d = x.shape
    assert d == D and B * L == N_CORES * S, (x.shape,)

    use_b1 = bool(np.any(b1 != 0.0))
    use_b2 = bool(np.any(b2 != 0.0))
    use_lng = bool(np.any(ln_g != 1.0))
    use_lnb = bool(np.any(ln_b != 0.0))

    key = (use_b2, use_lng, use_lnb, use_b1)
    if key not in _CACHE:
        _CACHE[key] = _build(*key[:3], use_b1=use_b1)
    nc = _CACHE[key]

    in_maps = prep_inputs(
        {"x": x, "w1": w1, "b1": b1, "w2": w2, "b2": b2, "ln_g": ln_g, "ln_b": ln_b}
    )

    res = run_bass_kernel_spmd(nc, in_maps, list(range(N_CORES)))
    out = np.concatenate([res.results[c]["out"] for c in range(N_CORES)], axis=0)
    return out.reshape(B, L, D).astype(np.float32)
